# revision 1
# baseline (speedup 1.0000x reference)
"""Birth-death loss kernel v2 for 8 TRN2 NeuronCores.

Per core (2 batches): endpoints are fetched with chunked dma_gather
(256-byte blocks of 64 f32), the wanted element of each block is picked
with an iota/is_equal mask + reduce on DVE, then (birth-death)^2 is
reduced. Good-interval flip handled via tiny static correction slices.

Host prep is pure layout: interval (i,j) pairs are pre-arranged into the
16-wrapped replicated int16 layout dma_gather requires; j is also sent
128-wrapped for the in-block select. All arithmetic (block index, j&63,
squares, sums) happens on device.

Endpoint order per stream (G=batch, T=interval tensor):
  k = e*32768 + c*8192 + n   (e: 0 birth / 1 death, c: class, n: interval)
"""

import numpy as np

import concourse.bass as bass
import concourse.bacc as bacc
import concourse.mybir as mybir
from concourse import library_config
from concourse.bass_utils import run_bass_kernel_spmd

B, C, H, W, N = 16, 4, 512, 512, 8192
NCORES = 8
BS = B // NCORES               # 2 batches/core
PRED_SZ = BS * C * H * W       # 2097152
G0 = (1, 1, 2, 1)
G1 = (0, 1, 0, 2)
NGOOD = BS * (sum(G0) + sum(G1))

NSTREAM = 4                    # (G, T) pairs: (0,0),(0,1),(1,0),(1,1)
KS = C * N * 2                 # endpoints per stream = 65536
CHUNK = 1024                   # endpoints per dma_gather call
NCH = KS // CHUNK              # 8 chunks per stream
VB = 4                         # gather buffers

f32 = mybir.dt.float32
i32 = mybir.dt.int32
i16 = mybir.dt.int16
Alu = mybir.AluOpType
X = mybir.AxisListType.X

STREAMS = [(g, t) for g in range(BS) for t in range(2)]
CNT = {0: G0, 1: G1}


def _build_nc():
    nc = bacc.Bacc(
        "TRN2", target_bir_lowering=False, debug=False, num_devices=NCORES,
        dynamic_dma_scratch_size=3 * 2**15, detect_race_conditions=False,
    )

    pred = nc.dram_tensor("pred", [PRED_SZ // 64, 64], f32, kind="ExternalInput").ap()
    d_a = [
        nc.dram_tensor(f"a{s}", [128, KS // 16 * 2], i16, kind="ExternalInput").ap()
        for s in range(NSTREAM)
    ]
    d_j = [
        nc.dram_tensor(f"j{s}", [128, KS // 128], i16, kind="ExternalInput").ap()
        for s in range(NSTREAM)
    ]
    d_iota = nc.dram_tensor("iotaf", [128, 64], f32, kind="ExternalInput").ap()
    outd = nc.dram_tensor("out", [1, 1], f32, kind="ExternalOutput").ap()

    sb_a = [nc.alloc_sbuf_tensor(f"sb_a{v}", [128, KS // 16 * 2], i16).ap()
            for v in range(2)]
    sb_j = [nc.alloc_sbuf_tensor(f"sb_j{s}", [128, KS // 128], i16).ap()
            for s in range(NSTREAM)]
    sb_wf = [nc.alloc_sbuf_tensor(f"sb_wf{s}", [128, KS // 128], f32).ap()
             for s in range(NSTREAM)]
    sb_idx = [nc.alloc_sbuf_tensor(f"sb_idx{s}", [128, KS // 16], i16).ap()
              for s in range(NSTREAM)]
    sb_iota = nc.alloc_sbuf_tensor("sb_iota", [128, 64], f32).ap()
    sb_t16d = nc.alloc_sbuf_tensor("sb_t16d", [128, KS // 16], i16).ap()
    sb_V = [nc.alloc_sbuf_tensor(f"sb_V{v}", [128, CHUNK // 128 * 64], f32).ap()
            for v in range(VB)]
    sb_M = [nc.alloc_sbuf_tensor(f"sb_M{v}", [128, CHUNK // 128 * 64], f32).ap()
            for v in range(2)]
    sb_VM = [nc.alloc_sbuf_tensor(f"sb_VM{v}", [128, CHUNK // 128 * 64], f32).ap()
             for v in range(2)]
    sb_sel = [nc.alloc_sbuf_tensor(f"sb_sel{s}", [128, KS // 128], f32).ap()
              for s in range(NSTREAM)]
    sb_d = [nc.alloc_sbuf_tensor(f"sb_d{s}", [128, KS // 256], f32).ap()
            for s in range(NSTREAM)]
    sb_part = nc.alloc_sbuf_tensor("sb_part", [128, 32], f32).ap()
    sb_S = nc.alloc_sbuf_tensor("sb_S", [128, 1], f32).ap()
    sb_ones = nc.alloc_sbuf_tensor("sb_ones", [128, 1], f32).ap()
    sb_res = nc.alloc_sbuf_tensor("sb_res", [1, 1], f32).ap()
    ps_out = nc.alloc_psum_tensor("ps_out", [1, 1], f32).ap()

    with (
        nc.Block() as block,
        nc.semaphore("dma_in") as dma_in,
        nc.semaphore("dma_a0") as dma_a0,
        nc.semaphore("dma_a1") as dma_a1,
        nc.semaphore("dma_a2") as dma_a2,
        nc.semaphore("dma_a3") as dma_a3,
        nc.semaphore("vv") as vv,
        nc.semaphore("idx_rdy") as idx_rdy,
        nc.semaphore("gat") as gat,
        nc.semaphore("vfree") as vfree,
        nc.semaphore("v_done") as v_done,
        nc.semaphore("t_done") as t_done,
    ):

        @block.sync
        def _(sy):
            dma_as = [dma_a0, dma_a1, dma_a2, dma_a3]
            for s in range(NSTREAM):
                if s >= 2:
                    sy.wait_ge(idx_rdy, s - 1)
                sy.dma_start(out=sb_a[s % 2], in_=d_a[s]).then_inc(dma_as[s], 16)
            for s in range(NSTREAM):
                sy.dma_start(out=sb_j[s], in_=d_j[s]).then_inc(dma_in, 16)
            sy.dma_start(out=sb_iota, in_=d_iota).then_inc(dma_in, 16)
            sy.wait_ge(v_done, 2)
            sy.dma_start(out=outd, in_=sb_res).then_inc(dma_in, 16)

        @block.vector
        def _(v):
            vc = [0]

            def S(ins):
                vc[0] += 1
                ins.then_inc(vv, 1)
                v.wait_ge(vv, vc[0])
                return ins

            # block indices per stream: blk = i2*8 + (j>>6), int16,
            # already in the wrapped+replicated layout
            dma_as = [dma_a0, dma_a1, dma_a2, dma_a3]
            for s in range(NSTREAM):
                v.wait_ge(dma_as[s], 16)
                av = sb_a[s % 2].rearrange("p (n x) -> p n x", x=2)
                sb_t16 = sb_t16d
                i2 = av[:, :, 0]
                jw = av[:, :, 1]
                S(v.tensor_scalar(sb_t16, jw, 6, None, Alu.logical_shift_right))
                v.scalar_tensor_tensor(
                    sb_idx[s], i2, 8, sb_t16, Alu.mult, Alu.add
                ).then_inc(idx_rdy, 1)
                v.wait_ge(idx_rdy, s + 1)
            # w = j&63 as f32, 128-wrapped (for the select mask)
            v.wait_ge(dma_in, 16 * (NSTREAM + 1))
            for s in range(NSTREAM):
                S(v.tensor_scalar(sb_j[s], sb_j[s], 63, None, Alu.bitwise_and))
                S(v.tensor_copy(sb_wf[s], sb_j[s]))
            v.memset(sb_ones, 1.0)
            S(v.memset(sb_part, 0.0))

            # chunk pipeline (per cg): wait gather -> mult (uses mask built
            # in the prior iteration) -> build next mask -> reduce.  The mask
            # build between mult and reduce doubles as the mult's pipeline
            # drain; M and VM are double-buffered so no same-buffer WAR.
            GPC = CHUNK // 128
            iota_bc = sb_iota.rearrange("p (o e) -> p o e", o=1).broadcast_to(
                [128, GPC, 64]
            )

            def mask_for(cg2):
                s2, c2 = divmod(cg2, NCH)
                wf_sl = sb_wf[s2][:, c2 * GPC:(c2 + 1) * GPC]
                w_bc = wf_sl.unsqueeze(-1).broadcast_to([128, GPC, 64])
                mv = sb_M[cg2 % 2].rearrange("p (n e) -> p n e", e=64)
                return v.tensor_tensor(mv, iota_bc, w_bc, Alu.is_equal)

            NTOT = NSTREAM * NCH
            S(mask_for(0))
            for cg in range(NTOT):
                s, c = divmod(cg, NCH)
                buf = sb_V[cg % VB].rearrange("p (n e) -> p n e", e=64)
                mv = sb_M[cg % 2].rearrange("p (n e) -> p n e", e=64)
                vmv = sb_VM[cg % 2].rearrange("p (n e) -> p n e", e=64)
                v.wait_ge(gat, 16 * (cg + 1))
                v.tensor_tensor(vmv, buf, mv, Alu.mult).then_inc(vfree, 1)
                if cg + 1 < NTOT:
                    mask_for(cg + 1)  # gap op = drain for the mult
                else:
                    v.wait_ge(vfree, NTOT)  # drain the last mult
                red = v.tensor_reduce(
                    sb_sel[s][:, c * GPC:(c + 1) * GPC], vmv, axis=X, op=Alu.add
                )
                if cg + 1 == NTOT:
                    S(red)

            # per stream: d = birth - death ; sum d^2 ; good corrections
            half = KS // 256  # 256 sel cols per stream; half = 256
            ccol = [NSTREAM]
            for s in range(NSTREAM):
                g, t = STREAMS[s]
                S(v.tensor_tensor(
                    sb_d[s], sb_sel[s][:, 0:half], sb_sel[s][:, half:2 * half],
                    Alu.subtract,
                ))
                S(v.tensor_tensor(sb_sel[s][:, 0:half], sb_d[s], sb_d[s], Alu.mult))
                S(v.tensor_reduce(
                    sb_part[:, s:s + 1], sb_sel[s][:, 0:half], axis=X, op=Alu.add
                ))
                for c4 in range(C):
                    cnt = CNT[t][c4]
                    if cnt == 0:
                        continue
                    dsl = sb_d[s][0:cnt, 64 * c4:64 * c4 + 1]
                    S(v.scalar_tensor_tensor(
                        sb_part[0:cnt, ccol[0]:ccol[0] + 1], dsl, -2.0, dsl,
                        Alu.mult, Alu.mult,
                    ))
                    ccol[0] += 1
            v.tensor_reduce(sb_S, sb_part, axis=X, op=Alu.add).then_inc(v_done, 1)

            v.wait_ge(t_done, 1)
            v.tensor_scalar(
                sb_res, ps_out, float(NGOOD), None, Alu.add
            ).then_inc(v_done, 1)

        @block.gpsimd
        def _(g):
            from concourse import library_config
            g.load_library(library_config.mlp)
            nidx_reg = g.alloc_register("nidx")
            g.reg_mov(nidx_reg, CHUNK)
            for cg in range(NSTREAM * NCH):
                s, c = divmod(cg, NCH)
                grp = STREAMS[s][0]
                g.wait_ge(idx_rdy, s + 1)
                if cg >= VB:
                    g.wait_ge(vfree, cg - VB + 1)
                src = pred[grp * (PRED_SZ // 128):(grp + 1) * (PRED_SZ // 128), :]
                g.dma_gather(
                    out_ap=sb_V[cg % VB].rearrange("p (n e) -> p n e", e=64),
                    in_ap=src,
                    idxs_ap=sb_idx[s][:, c * (CHUNK // 16):(c + 1) * (CHUNK // 16)],
                    num_idxs=CHUNK,
                    num_idxs_reg=nidx_reg,
                    elem_size=64,
                ).then_inc(gat, 16)

        @block.tensor
        def _(te):
            te.wait_ge(v_done, 1)
            te.matmul(ps_out, sb_S, sb_ones, start=True, stop=True).then_inc(
                t_done, 1
            )

    nc.compile()
    return nc


_NC = None


def _get_nc():
    global _NC
    if _NC is None:
        _NC = _build_nc()
    return _NC


def _host_prep(iv, t):
    """iv: (BS, C, N, 2, 2) int32 for interval tensor t.
    Returns per-group (a16 [128, KS//16*2], j128 [128, KS//128]) lists."""
    outs = []
    for g in range(BS):
        i = iv[g, :, :, :, 0].astype(np.int32)   # (C, N, 2)
        j = iv[g, :, :, :, 1].astype(np.int32)
        i2 = i + 512 * np.arange(C, dtype=np.int32)[:, None, None]
        # k-order: (e, c, n)
        i2k = np.transpose(i2, (2, 0, 1)).reshape(KS)
        jk = np.transpose(j, (2, 0, 1)).reshape(KS)
        pair = np.stack([i2k, jk], axis=-1).astype(np.int16)   # (KS, 2)
        wrapped = pair.reshape(KS // 16, 16, 2).transpose(1, 0, 2).reshape(
            16, KS // 16 * 2
        )
        a16 = np.tile(wrapped, (8, 1))
        j128 = jk.reshape(KS // 128, 128).T.astype(np.int16).copy()
        outs.append((a16, j128))
    return outs


def make_in_maps(prediction, intervals_comp_0, intervals_comp_1):
    iotaf = np.tile(np.arange(64, dtype=np.float32), (128, 1))
    in_maps = []
    for m in range(NCORES):
        sl = slice(m * BS, (m + 1) * BS)
        predc = np.ascontiguousarray(prediction[sl], dtype=np.float32).reshape(
            PRED_SZ // 64, 64
        )
        prep = {0: _host_prep(np.asarray(intervals_comp_0[sl]), 0),
                1: _host_prep(np.asarray(intervals_comp_1[sl]), 1)}
        im = {"pred": predc, "iotaf": iotaf}
        for s, (g, t) in enumerate(STREAMS):
            a16, j128 = prep[t][g]
            im[f"a{s}"] = a16
            im[f"j{s}"] = j128
        in_maps.append(im)
    return in_maps


def kernel(prediction, intervals_comp_0, intervals_comp_1, **run_kwargs):
    nc = _get_nc()
    in_maps = make_in_maps(prediction, intervals_comp_0, intervals_comp_1)
    res = run_bass_kernel_spmd(nc, in_maps, list(range(NCORES)), **run_kwargs)
    total = np.float32(0.0)
    for r in res.results:
        total += np.float32(r["out"].reshape(())[()])
    kernel.last_result = res
    return np.array(total, dtype=np.float32)



# revision 15
# speedup vs baseline: 1.2889x; 1.2889x over previous
"""Birth-death loss kernel v6 for 8 TRN2 NeuronCores.

Per core (2 batches): endpoints are fetched with chunked dma_gather
(256-byte blocks of 64 f32, the hardware minimum; the gather ucode caps
num_idxs at 1024 per call).  Selection of the wanted element from each
block is split across three engines so no engine exceeds the DMA
transfer rate (1.46us per 1024-endpoint chunk):

  - PE builds D = w - iota into an alternating PSUM bank with two
    accumulating broadcast matmuls (identity x w_bc, minus-ones-row x
    iota_bc), ~0.9us/chunk on an otherwise idle engine.
  - DVE fuses mask+multiply into one scalar_tensor_tensor
    ((0 is_equal D) mult V) and does the per-block reduce: 2 ops,
    ~1.25us/chunk.
  - Pool (GPSIMD) only generates gather descriptors, ~1.38us/chunk.

Host prep ships the block index (k>>6, int16, 16-wrapped x8 replicated,
the dma_gather format) and the in-block offset (j&63 as bf16, packed) -
pure index layout, no float math on host.

Endpoint order per stream (g=batch-in-core, t=interval tensor):
  k = e*32768 + c*8192 + n   (e: 0 birth / 1 death, c: class, n: interval)
"""

import numpy as np

import concourse.bass as bass
import concourse.bacc as bacc
import concourse.mybir as mybir
from concourse import library_config
from concourse.bass_utils import run_bass_kernel_spmd

B, C, H, W, N = 16, 4, 512, 512, 8192
NCORES = 8
BS = B // NCORES               # 2 batches/core
PRED_SZ = BS * C * H * W       # 2097152
G0 = (1, 1, 2, 1)
G1 = (0, 1, 0, 2)
NGOOD = BS * (sum(G0) + sum(G1))

NSTREAM = 4                    # (g, t) pairs: (0,0),(0,1),(1,0),(1,1)
KS = C * N * 2                 # endpoints per stream = 65536
CHUNK = 1024                   # endpoints per dma_gather call (ucode max)
NCH = KS // CHUNK              # 64 chunks per stream
NTOT = NSTREAM * NCH           # 256 chunks
GPC = CHUNK // 128             # 8 block-columns per chunk
VB = 6                         # gather buffers
WB = 2                         # VM buffers

f32 = mybir.dt.float32
bf16 = mybir.dt.bfloat16
i16 = mybir.dt.int16
Alu = mybir.AluOpType
X = mybir.AxisListType.X

STREAMS = [(g, t) for g in range(BS) for t in range(2)]
CNT = {0: G0, 1: G1}
HALF = KS // 256               # 256 sel cols per stream half

# load order: blk0a, blk0b, iota, moh, iden, w0, blk1, w1, blk2, w2, blk3, w3
BLK_RDY = [32, 112, 144, 176]
W_RDY = [96, 128, 160, 192]


def _build_nc():
    nc = bacc.Bacc(
        "TRN2", target_bir_lowering=False, debug=False, num_devices=NCORES,
        dynamic_dma_scratch_size=3 * 2**15, detect_race_conditions=False,
    )

    pred = nc.dram_tensor("pred", [PRED_SZ // 64, 64], f32, kind="ExternalInput").ap()
    d_blk = [nc.dram_tensor(f"blk{s}", [128, KS // 16], i16, kind="ExternalInput").ap()
             for s in range(NSTREAM)]
    d_w = [nc.dram_tensor(f"w{s}", [128, KS // 128], bf16, kind="ExternalInput").ap()
           for s in range(NSTREAM)]
    d_iota = nc.dram_tensor("iotaf", [1, 64], bf16, kind="ExternalInput").ap()
    d_iden = nc.dram_tensor("iden", [128, 128], bf16, kind="ExternalInput").ap()
    d_moh = nc.dram_tensor("moh", [1, 128], bf16, kind="ExternalInput").ap()
    outd = nc.dram_tensor("out", [1, 1], f32, kind="ExternalOutput").ap()

    sb_blk = [nc.alloc_sbuf_tensor(f"sb_blk{s}", [128, KS // 16], i16).ap()
              for s in range(NSTREAM)]
    sb_w = [nc.alloc_sbuf_tensor(f"sb_w{s}", [128, KS // 128], bf16).ap()
            for s in range(NSTREAM)]
    sb_iota = nc.alloc_sbuf_tensor("sb_iota", [1, 64], bf16).ap()
    sb_iden = nc.alloc_sbuf_tensor("sb_iden", [128, 128], bf16).ap()
    sb_moh = nc.alloc_sbuf_tensor("sb_moh", [1, 128], bf16).ap()
    sb_V = [nc.alloc_sbuf_tensor(f"sb_V{v}", [128, GPC * 64], f32).ap()
            for v in range(VB)]
    sb_VM = [nc.alloc_sbuf_tensor(f"sb_VM{v}", [128, GPC * 64], f32).ap()
             for v in range(WB)]
    sb_sel = [nc.alloc_sbuf_tensor(f"sb_sel{s}", [128, KS // 128], f32).ap()
              for s in range(NSTREAM)]
    sb_d = [nc.alloc_sbuf_tensor(f"sb_d{s}", [128, HALF], f32).ap()
            for s in range(NSTREAM)]
    sb_sq = nc.alloc_sbuf_tensor("sb_sq", [128, HALF], f32).ap()
    sb_part = nc.alloc_sbuf_tensor("sb_part", [128, 32], f32).ap()
    sb_S = nc.alloc_sbuf_tensor("sb_S", [128, 1], f32).ap()
    sb_ones = nc.alloc_sbuf_tensor("sb_ones", [128, 1], f32).ap()
    sb_res = nc.alloc_sbuf_tensor("sb_res", [1, 1], f32).ap()
    ps_D = [nc.alloc_psum_tensor(f"ps_D{v}", [128, GPC * 64], f32).ap()
            for v in range(2)]
    ps_out = nc.alloc_psum_tensor("ps_out", [1, 1], f32).ap()

    with (
        nc.Block() as block,
        nc.semaphore("dma_in") as dma_in,
        nc.semaphore("gat") as gat,
        nc.semaphore("ped") as ped,        # PE: D bank ready
        nc.semaphore("sttd") as sttd,      # DVE STT done: V free, D bank free
        nc.semaphore("vt") as vt,          # tail-op drain chain
        nc.semaphore("v_done") as v_done,
        nc.semaphore("t_done") as t_done,
    ):

        @block.sync
        def _(sy):
            nch0 = CHUNK // 16
            sy.dma_start(out=sb_blk[0][:, 0:nch0], in_=d_blk[0][:, 0:nch0]
                         ).then_inc(dma_in, 16)
            sy.dma_start(out=sb_blk[0][:, nch0:], in_=d_blk[0][:, nch0:]
                         ).then_inc(dma_in, 16)
            sy.dma_start(out=sb_iota, in_=d_iota).then_inc(dma_in, 16)
            sy.dma_start(out=sb_moh, in_=d_moh).then_inc(dma_in, 16)
            sy.dma_start(out=sb_iden, in_=d_iden).then_inc(dma_in, 16)
            sy.dma_start(out=sb_w[0], in_=d_w[0]).then_inc(dma_in, 16)
            for s in range(1, NSTREAM):
                sy.dma_start(out=sb_blk[s], in_=d_blk[s]).then_inc(dma_in, 16)
                sy.dma_start(out=sb_w[s], in_=d_w[s]).then_inc(dma_in, 16)
            sy.wait_ge(v_done, 2)
            sy.dma_start(out=outd, in_=sb_res).then_inc(dma_in, 16)

        @block.gpsimd
        def _(g):
            g.load_library(library_config.mlp)
            nidx_reg = g.alloc_register("nidx")
            g.reg_mov(nidx_reg, CHUNK)
            for cg in range(NTOT):
                s, c = divmod(cg, NCH)
                grp = STREAMS[s][0]
                if cg == 0:
                    g.wait_ge(dma_in, 16)
                elif c == 0 or cg == 1:
                    g.wait_ge(dma_in, BLK_RDY[s])
                if cg >= VB:
                    g.wait_ge(sttd, cg - VB + 1)
                src = pred[grp * (PRED_SZ // 128):(grp + 1) * (PRED_SZ // 128), :]
                g.dma_gather(
                    out_ap=sb_V[cg % VB].rearrange("p (n e) -> p n e", e=64),
                    in_ap=src,
                    idxs_ap=sb_blk[s][:, c * (CHUNK // 16):(c + 1) * (CHUNK // 16)],
                    num_idxs=CHUNK,
                    num_idxs_reg=nidx_reg,
                    elem_size=64,
                ).then_inc(gat, 16)

        @block.tensor
        def _(te):
            iota_bc = sb_iota.rearrange("o (g e) -> o g e", g=1).broadcast_to(
                [1, GPC, 64]
            )
            te.wait_ge(dma_in, W_RDY[0])
            for cg in range(NTOT):
                s, c = divmod(cg, NCH)
                if c == 0 and cg > 0:
                    te.wait_ge(dma_in, W_RDY[s])
                if cg >= 2:
                    te.wait_ge(sttd, cg - 1)
                w_bc = sb_w[s][:, c * GPC:(c + 1) * GPC].unsqueeze(-1
                    ).broadcast_to([128, GPC, 64])
                psv = ps_D[cg % 2].rearrange("p (g e) -> p g e", e=64)
                te.matmul(psv, sb_iden, w_bc, start=True, stop=False)
                te.matmul(psv, sb_moh, iota_bc, start=False, stop=True
                          ).then_inc(ped, 1)
            te.wait_ge(v_done, 1)
            te.matmul(ps_out, sb_S, sb_ones, start=True, stop=True).then_inc(
                t_done, 1
            )

        @block.vector
        def _(v):
            tc = [0]

            def T(ins):
                tc[0] += 1
                ins.then_inc(vt, 1)
                v.wait_ge(vt, tc[0])
                return ins

            def reduce_for(k):
                s2, c2 = divmod(k, NCH)
                vmv = sb_VM[k % WB].rearrange("p (n e) -> p n e", e=64)
                v.tensor_reduce(
                    sb_sel[s2][:, c2 * GPC:(c2 + 1) * GPC], vmv, axis=X,
                    op=Alu.add,
                )

            def tail_for(s2):
                T(v.tensor_tensor(
                    sb_d[s2], sb_sel[s2][:, 0:HALF],
                    sb_sel[s2][:, HALF:2 * HALF], Alu.subtract,
                ))

            def tail2_for(s2):
                T(v.tensor_tensor(sb_sq, sb_d[s2], sb_d[s2], Alu.mult))
                T(v.tensor_reduce(
                    sb_part[:, s2:s2 + 1], sb_sq, axis=X, op=Alu.add,
                ))
                t = STREAMS[s2][1]
                for c4 in range(C):
                    cnt = CNT[t][c4]
                    if cnt == 0:
                        continue
                    dsl = sb_d[s2][0:cnt, 64 * c4:64 * c4 + 1]
                    v.scalar_tensor_tensor(
                        sb_part[0:cnt, 4 + 4 * s2 + c4:5 + 4 * s2 + c4], dsl,
                        -2.0, dsl, Alu.mult, Alu.mult,
                    )

            v.memset(sb_part, 0.0)
            v.memset(sb_ones, 1.0)
            for cg in range(NTOT):
                buf = sb_V[cg % VB].rearrange("p (n e) -> p n e", e=64)
                vmv = sb_VM[cg % WB].rearrange("p (n e) -> p n e", e=64)
                dv = ps_D[cg % 2].rearrange("p (g e) -> p g e", e=64)
                v.wait_ge(gat, 16 * (cg + 1))
                v.wait_ge(ped, cg + 1)
                # VM = (0 == D) * V   (fused mask+select)
                v.scalar_tensor_tensor(
                    vmv, dv, 0.0, buf, Alu.is_equal, Alu.mult
                ).then_inc(sttd, 1)
                if cg >= 1:
                    reduce_for(cg - 1)     # gap op for the STT
                for s2 in range(NSTREAM - 1):
                    if cg == (s2 + 1) * NCH + 2:
                        tail_for(s2)
                    elif cg == (s2 + 1) * NCH + 4:
                        tail2_for(s2)
            reduce_for(NTOT - 1)
            tail_for(3)
            tail2_for(3)
            T(v.memset(sb_sq[0:1, 0:1], 0.0))   # drain corrections
            v.tensor_reduce(sb_S, sb_part, axis=X, op=Alu.add).then_inc(
                v_done, 1
            )

            v.wait_ge(t_done, 1)
            v.tensor_scalar(
                sb_res, ps_out, float(NGOOD), None, Alu.add
            ).then_inc(v_done, 1)

    nc.compile()
    return nc


_NC = None


def _get_nc():
    global _NC
    if _NC is None:
        _NC = _build_nc()
    return _NC


def _host_prep(iv):
    """iv: (C, N, 2, 2) int32 for one (group, tensor) stream.
    Returns (blk16 [128, KS//16] int16 16-wrapped x8, w [128, KS//128] bf16)."""
    import ml_dtypes
    i = iv[:, :, :, 0].astype(np.int32)   # (C, N, 2)
    j = iv[:, :, :, 1].astype(np.int32)
    i2 = i + 512 * np.arange(C, dtype=np.int32)[:, None, None]
    # k-order: (e, c, n)
    i2k = np.transpose(i2, (2, 0, 1)).reshape(KS)
    jk = np.transpose(j, (2, 0, 1)).reshape(KS)
    blk = ((i2k << 3) | (jk >> 6)).astype(np.int16)
    blk16 = np.tile(blk.reshape(KS // 16, 16).T, (8, 1))
    w = (jk & 63).astype(ml_dtypes.bfloat16).reshape(KS // 128, 128).T.copy()
    return blk16, w


def make_in_maps(prediction, intervals_comp_0, intervals_comp_1):
    import ml_dtypes
    iotaf = np.arange(64, dtype=ml_dtypes.bfloat16).reshape(1, 64)
    iden = np.eye(128, dtype=ml_dtypes.bfloat16)
    moh = np.full((1, 128), -1, dtype=ml_dtypes.bfloat16)
    ivs = {0: intervals_comp_0, 1: intervals_comp_1}
    in_maps = []
    for m in range(NCORES):
        sl = slice(m * BS, (m + 1) * BS)
        predc = np.ascontiguousarray(prediction[sl], dtype=np.float32).reshape(
            PRED_SZ // 64, 64
        )
        im = {"pred": predc, "iotaf": iotaf, "iden": iden, "moh": moh}
        for s, (g, t) in enumerate(STREAMS):
            blk16, w = _host_prep(np.asarray(ivs[t][sl][g]))
            im[f"blk{s}"] = blk16
            im[f"w{s}"] = w
        in_maps.append(im)
    return in_maps


def kernel(prediction, intervals_comp_0, intervals_comp_1, **run_kwargs):
    nc = _get_nc()
    in_maps = make_in_maps(prediction, intervals_comp_0, intervals_comp_1)
    res = run_bass_kernel_spmd(nc, in_maps, list(range(NCORES)), **run_kwargs)
    total = np.float32(0.0)
    for r in res.results:
        total += np.float32(r["out"].reshape(())[()])
    kernel.last_result = res
    return np.array(total, dtype=np.float32)


# revision 16
# speedup vs baseline: 1.2892x; 1.0002x over previous
"""Birth-death loss kernel v6 for 8 TRN2 NeuronCores.

Per core (2 batches): endpoints are fetched with chunked dma_gather
(256-byte blocks of 64 f32, the hardware minimum; the gather ucode caps
num_idxs at 1024 per call).  Selection of the wanted element from each
block is split across three engines so no engine exceeds the DMA
transfer rate (1.46us per 1024-endpoint chunk):

  - PE builds D = w - iota into an alternating PSUM bank with two
    accumulating broadcast matmuls (identity x w_bc, minus-ones-row x
    iota_bc), ~0.9us/chunk on an otherwise idle engine.
  - DVE fuses mask+multiply into one scalar_tensor_tensor
    ((0 is_equal D) mult V) and does the per-block reduce: 2 ops,
    ~1.25us/chunk.
  - Pool (GPSIMD) only generates gather descriptors, ~1.38us/chunk.

Host prep ships the block index (k>>6, int16, 16-wrapped x8 replicated,
the dma_gather format) and the in-block offset (j&63 as bf16, packed) -
pure index layout, no float math on host.

Endpoint order per stream (g=batch-in-core, t=interval tensor):
  k = e*32768 + c*8192 + n   (e: 0 birth / 1 death, c: class, n: interval)
"""

import numpy as np

import concourse.bass as bass
import concourse.bacc as bacc
import concourse.mybir as mybir
from concourse import library_config
from concourse.bass_utils import run_bass_kernel_spmd

B, C, H, W, N = 16, 4, 512, 512, 8192
NCORES = 8
BS = B // NCORES               # 2 batches/core
PRED_SZ = BS * C * H * W       # 2097152
G0 = (1, 1, 2, 1)
G1 = (0, 1, 0, 2)
NGOOD = BS * (sum(G0) + sum(G1))

NSTREAM = 4                    # (g, t) pairs: (0,0),(0,1),(1,0),(1,1)
KS = C * N * 2                 # endpoints per stream = 65536
CHUNK = 1024                   # endpoints per dma_gather call (ucode max)
NCH = KS // CHUNK              # 64 chunks per stream
NTOT = NSTREAM * NCH           # 256 chunks
GPC = CHUNK // 128             # 8 block-columns per chunk
VB = 6                         # gather buffers
WB = 2                         # VM buffers

f32 = mybir.dt.float32
bf16 = mybir.dt.bfloat16
i16 = mybir.dt.int16
Alu = mybir.AluOpType
X = mybir.AxisListType.X

STREAMS = [(g, t) for g in range(BS) for t in range(2)]
CNT = {0: G0, 1: G1}
HALF = KS // 256               # 256 sel cols per stream half

# load order: blk0a, blk0b, iota, moh, iden, w0, blk1, w1, blk2, w2, blk3, w3
BLK_RDY = [32, 112, 144, 176]
W_RDY = [96, 128, 160, 192]


def _build_nc():
    nc = bacc.Bacc(
        "TRN2", target_bir_lowering=False, debug=False, num_devices=NCORES,
        dynamic_dma_scratch_size=3 * 2**15, detect_race_conditions=False,
    )

    pred = nc.dram_tensor("pred", [PRED_SZ // 64, 64], f32, kind="ExternalInput").ap()
    d_blk = [nc.dram_tensor(f"blk{s}", [128, KS // 16], i16, kind="ExternalInput").ap()
             for s in range(NSTREAM)]
    d_w = [nc.dram_tensor(f"w{s}", [128, KS // 128], bf16, kind="ExternalInput").ap()
           for s in range(NSTREAM)]
    d_iota = nc.dram_tensor("iotaf", [1, 64], bf16, kind="ExternalInput").ap()
    d_iden = nc.dram_tensor("iden", [128, 128], bf16, kind="ExternalInput").ap()
    d_moh = nc.dram_tensor("moh", [1, 128], bf16, kind="ExternalInput").ap()
    outd = nc.dram_tensor("out", [1, 1], f32, kind="ExternalOutput").ap()

    sb_blk = [nc.alloc_sbuf_tensor(f"sb_blk{s}", [128, KS // 16], i16).ap()
              for s in range(NSTREAM)]
    sb_w = [nc.alloc_sbuf_tensor(f"sb_w{s}", [128, KS // 128], bf16).ap()
            for s in range(NSTREAM)]
    sb_iota = nc.alloc_sbuf_tensor("sb_iota", [1, 64], bf16).ap()
    sb_iden = nc.alloc_sbuf_tensor("sb_iden", [128, 128], bf16).ap()
    sb_moh = nc.alloc_sbuf_tensor("sb_moh", [1, 128], bf16).ap()
    sb_V = [nc.alloc_sbuf_tensor(f"sb_V{v}", [128, GPC * 64], f32).ap()
            for v in range(VB)]
    sb_VM = [nc.alloc_sbuf_tensor(f"sb_VM{v}", [128, GPC * 64], f32).ap()
             for v in range(WB)]
    sb_sel = [nc.alloc_sbuf_tensor(f"sb_sel{s}", [128, KS // 128], f32).ap()
              for s in range(NSTREAM)]
    sb_d = [nc.alloc_sbuf_tensor(f"sb_d{s}", [128, HALF], f32).ap()
            for s in range(NSTREAM)]
    sb_sq = nc.alloc_sbuf_tensor("sb_sq", [128, HALF], f32).ap()
    sb_part = nc.alloc_sbuf_tensor("sb_part", [128, 32], f32).ap()
    sb_res = nc.alloc_sbuf_tensor("sb_res", [1, 1], f32).ap()
    ps_D = [nc.alloc_psum_tensor(f"ps_D{v}", [128, GPC * 64], f32).ap()
            for v in range(2)]

    with (
        nc.Block() as block,
        nc.semaphore("dma_in") as dma_in,
        nc.semaphore("gat") as gat,
        nc.semaphore("ped") as ped,        # PE: D bank ready
        nc.semaphore("sttd") as sttd,      # DVE STT done: V free, D bank free
        nc.semaphore("vt") as vt,          # tail-op drain chain
        nc.semaphore("v_done") as v_done,
    ):

        @block.sync
        def _(sy):
            nch0 = CHUNK // 16
            sy.dma_start(out=sb_blk[0][:, 0:nch0], in_=d_blk[0][:, 0:nch0]
                         ).then_inc(dma_in, 16)
            sy.dma_start(out=sb_blk[0][:, nch0:], in_=d_blk[0][:, nch0:]
                         ).then_inc(dma_in, 16)
            sy.dma_start(out=sb_iota, in_=d_iota).then_inc(dma_in, 16)
            sy.dma_start(out=sb_moh, in_=d_moh).then_inc(dma_in, 16)
            sy.dma_start(out=sb_iden, in_=d_iden).then_inc(dma_in, 16)
            sy.dma_start(out=sb_w[0], in_=d_w[0]).then_inc(dma_in, 16)
            for s in range(1, NSTREAM):
                sy.dma_start(out=sb_blk[s], in_=d_blk[s]).then_inc(dma_in, 16)
                sy.dma_start(out=sb_w[s], in_=d_w[s]).then_inc(dma_in, 16)
            sy.wait_ge(v_done, 2)
            sy.dma_start(out=outd, in_=sb_res).then_inc(dma_in, 16)

        @block.gpsimd
        def _(g):
            g.load_library(library_config.mlp)
            nidx_reg = g.alloc_register("nidx")
            g.reg_mov(nidx_reg, CHUNK)
            for cg in range(NTOT):
                s, c = divmod(cg, NCH)
                grp = STREAMS[s][0]
                if cg == 0:
                    g.wait_ge(dma_in, 16)
                elif c == 0 or cg == 1:
                    g.wait_ge(dma_in, BLK_RDY[s])
                if cg >= VB:
                    g.wait_ge(sttd, cg - VB + 1)
                src = pred[grp * (PRED_SZ // 128):(grp + 1) * (PRED_SZ // 128), :]
                g.dma_gather(
                    out_ap=sb_V[cg % VB].rearrange("p (n e) -> p n e", e=64),
                    in_ap=src,
                    idxs_ap=sb_blk[s][:, c * (CHUNK // 16):(c + 1) * (CHUNK // 16)],
                    num_idxs=CHUNK,
                    num_idxs_reg=nidx_reg,
                    elem_size=64,
                ).then_inc(gat, 16)
            g.wait_ge(v_done, 1)
            g.tensor_reduce(sb_res, sb_part, axis=mybir.AxisListType.XYZWC,
                            op=Alu.add)
            g.tensor_scalar(sb_res, sb_res, float(NGOOD), None, Alu.add
                            ).then_inc(v_done, 1)

        @block.tensor
        def _(te):
            iota_bc = sb_iota.rearrange("o (g e) -> o g e", g=1).broadcast_to(
                [1, GPC, 64]
            )
            te.wait_ge(dma_in, W_RDY[0])
            for cg in range(NTOT):
                s, c = divmod(cg, NCH)
                if c == 0 and cg > 0:
                    te.wait_ge(dma_in, W_RDY[s])
                if cg >= 2:
                    te.wait_ge(sttd, cg - 1)
                w_bc = sb_w[s][:, c * GPC:(c + 1) * GPC].unsqueeze(-1
                    ).broadcast_to([128, GPC, 64])
                psv = ps_D[cg % 2].rearrange("p (g e) -> p g e", e=64)
                te.matmul(psv, sb_iden, w_bc, start=True, stop=False)
                te.matmul(psv, sb_moh, iota_bc, start=False, stop=True
                          ).then_inc(ped, 1)

        @block.vector
        def _(v):
            tc = [0]

            def T(ins):
                tc[0] += 1
                ins.then_inc(vt, 1)
                v.wait_ge(vt, tc[0])
                return ins

            def reduce_for(k):
                s2, c2 = divmod(k, NCH)
                vmv = sb_VM[k % WB].rearrange("p (n e) -> p n e", e=64)
                v.tensor_reduce(
                    sb_sel[s2][:, c2 * GPC:(c2 + 1) * GPC], vmv, axis=X,
                    op=Alu.add,
                )

            def tail_for(s2):
                T(v.tensor_tensor(
                    sb_d[s2], sb_sel[s2][:, 0:HALF],
                    sb_sel[s2][:, HALF:2 * HALF], Alu.subtract,
                ))

            def tail2_for(s2):
                T(v.tensor_tensor(sb_sq, sb_d[s2], sb_d[s2], Alu.mult))
                T(v.tensor_reduce(
                    sb_part[:, s2:s2 + 1], sb_sq, axis=X, op=Alu.add,
                ))
                t = STREAMS[s2][1]
                for c4 in range(C):
                    cnt = CNT[t][c4]
                    if cnt == 0:
                        continue
                    dsl = sb_d[s2][0:cnt, 64 * c4:64 * c4 + 1]
                    v.scalar_tensor_tensor(
                        sb_part[0:cnt, 4 + 4 * s2 + c4:5 + 4 * s2 + c4], dsl,
                        -2.0, dsl, Alu.mult, Alu.mult,
                    )

            v.memset(sb_part, 0.0)
            for cg in range(NTOT):
                buf = sb_V[cg % VB].rearrange("p (n e) -> p n e", e=64)
                vmv = sb_VM[cg % WB].rearrange("p (n e) -> p n e", e=64)
                dv = ps_D[cg % 2].rearrange("p (g e) -> p g e", e=64)
                v.wait_ge(gat, 16 * (cg + 1))
                v.wait_ge(ped, cg + 1)
                # VM = (0 == D) * V   (fused mask+select)
                v.scalar_tensor_tensor(
                    vmv, dv, 0.0, buf, Alu.is_equal, Alu.mult
                ).then_inc(sttd, 1)
                if cg >= 1:
                    reduce_for(cg - 1)     # gap op for the STT
                for s2 in range(NSTREAM - 1):
                    if cg == (s2 + 1) * NCH + 2:
                        tail_for(s2)
                    elif cg == (s2 + 1) * NCH + 4:
                        tail2_for(s2)
            reduce_for(NTOT - 1)
            tail_for(3)
            tail2_for(3)
            T(v.memset(sb_sq[0:1, 0:1], 0.0))   # drain corrections
            v.nop().then_inc(v_done, 1)

    nc.compile()
    return nc


_NC = None


def _get_nc():
    global _NC
    if _NC is None:
        _NC = _build_nc()
    return _NC


def _host_prep(iv):
    """iv: (C, N, 2, 2) int32 for one (group, tensor) stream.
    Returns (blk16 [128, KS//16] int16 16-wrapped x8, w [128, KS//128] bf16)."""
    import ml_dtypes
    i = iv[:, :, :, 0].astype(np.int32)   # (C, N, 2)
    j = iv[:, :, :, 1].astype(np.int32)
    i2 = i + 512 * np.arange(C, dtype=np.int32)[:, None, None]
    # k-order: (e, c, n)
    i2k = np.transpose(i2, (2, 0, 1)).reshape(KS)
    jk = np.transpose(j, (2, 0, 1)).reshape(KS)
    blk = ((i2k << 3) | (jk >> 6)).astype(np.int16)
    blk16 = np.tile(blk.reshape(KS // 16, 16).T, (8, 1))
    w = (jk & 63).astype(ml_dtypes.bfloat16).reshape(KS // 128, 128).T.copy()
    return blk16, w


def make_in_maps(prediction, intervals_comp_0, intervals_comp_1):
    import ml_dtypes
    iotaf = np.arange(64, dtype=ml_dtypes.bfloat16).reshape(1, 64)
    iden = np.eye(128, dtype=ml_dtypes.bfloat16)
    moh = np.full((1, 128), -1, dtype=ml_dtypes.bfloat16)
    ivs = {0: intervals_comp_0, 1: intervals_comp_1}
    in_maps = []
    for m in range(NCORES):
        sl = slice(m * BS, (m + 1) * BS)
        predc = np.ascontiguousarray(prediction[sl], dtype=np.float32).reshape(
            PRED_SZ // 64, 64
        )
        im = {"pred": predc, "iotaf": iotaf, "iden": iden, "moh": moh}
        for s, (g, t) in enumerate(STREAMS):
            blk16, w = _host_prep(np.asarray(ivs[t][sl][g]))
            im[f"blk{s}"] = blk16
            im[f"w{s}"] = w
        in_maps.append(im)
    return in_maps


def kernel(prediction, intervals_comp_0, intervals_comp_1, **run_kwargs):
    nc = _get_nc()
    in_maps = make_in_maps(prediction, intervals_comp_0, intervals_comp_1)
    res = run_bass_kernel_spmd(nc, in_maps, list(range(NCORES)), **run_kwargs)
    total = np.float32(0.0)
    for r in res.results:
        total += np.float32(r["out"].reshape(())[()])
    kernel.last_result = res
    return np.array(total, dtype=np.float32)
